# revision 9
# baseline (speedup 1.0000x reference)
"""Trainium2 Bass kernel: GatedRecurrentCell (v2.1, software-pipelined).

Math (per batch b, channels on partitions, time on free dim):
    w  = silu(pi + bi)                      [ACT, silu table]
    t  = tanh(pa/2 + ba/2)                  [ACT, same table set]
    a  = exp(-ln3/2 * t + (ln(sigmoid(g)) - ln3/2))   [ACT, exp table]
    a2 = a*a                                [GpSimd]
    q  = sqrt(1 - a2)                       [ACT, sqrt table]
    c  = q * w                              [DVE, all-bf16 2x]
    h  = scan(h = a*h + c), h0 = 0          [DVE tensor_tensor_scan]

Data-parallel over batch (8 cores, 1 batch each). GEMM inputs bf16 (the
GEMM rounding enters before the sigmoid, so the q = sqrt(1-a^2)
amplification ~a^2/q^2 does not see it); t/a/a2 stay fp32 (q would
amplify their rounding ~500x); w/q/c/h bf16 (enter h un-amplified).

Channel chunks (16 x 128) run in cycles of 3; ACT phases are
software-pipelined as P1(k+1) -> P2(k) -> P3(k): the PSUM-draining
phase P1 (silu+tanh, one table set) of the NEXT cycle executes between
the exp/sqrt phases of the current one, so the PE never waits a full
exp+sqrt window for PSUM (the v2.0 ping-pong). Table loads: 3/cycle.
"""

import functools
import os

import numpy as np

B, S, D, I = 8, 2048, 512, 2048
P = 128
NCORES = 8
LN3 = float(np.log(3.0))

MM_DT = os.environ.get("GRC_MM_DT", "bf16")      # "bf16" | "f32r"
CYC = int(os.environ.get("GRC_C", "2"))          # chunks per cycle
WC_ENGINE = os.environ.get("GRC_WC", "dve")      # c = q*w engine
A2_ENGINE = os.environ.get("GRC_A2", "gp")       # a2 = a*a engine
WQ_DT = os.environ.get("GRC_WQ_DT", "bf16")      # w/q dtype
CH_DT = os.environ.get("GRC_CH_DT", "bf16")      # c/h (scan in1/out) dtype


def _build_nc(s, d, i, mm_dt=MM_DT, cyc=CYC, wc_engine=WC_ENGINE,
              a2_engine=A2_ENGINE, wq_dt=WQ_DT, ch_dt=CH_DT, silu=True,
              has_bi=False):
    import concourse.bacc as bacc
    import concourse.mybir as mybir
    import concourse.tile as tile
    from concourse.tile import add_dep_helper

    F32 = mybir.dt.float32
    BF16 = mybir.dt.bfloat16
    AF = mybir.ActivationFunctionType
    ALU = mybir.AluOpType

    MMDT = BF16 if mm_dt == "bf16" else mybir.dt.float32r
    WQDT = BF16 if wq_dt == "bf16" else F32
    CHDT = BF16 if ch_dt == "bf16" else F32
    nd = d // P
    ni = i // P
    cyc = min(cyc, ni)
    nmm = s // 512

    nc = bacc.Bacc("TRN2", target_bir_lowering=False, debug=False,
                   num_devices=NCORES)

    xT_d = nc.dram_tensor("xT", [d, s], MMDT, kind="ExternalInput").ap()
    waT_d = nc.dram_tensor("WaT", [ni, P, d], MMDT, kind="ExternalInput").ap()
    wiT_d = nc.dram_tensor("WiT", [ni, P, d], MMDT, kind="ExternalInput").ap()
    bi_d = nc.dram_tensor("biT", [P, ni], F32, kind="ExternalInput").ap()
    bah_d = nc.dram_tensor("bahT", [P, ni], F32, kind="ExternalInput").ap()
    lnam_d = nc.dram_tensor("lnamT", [P, ni], F32, kind="ExternalInput").ap()
    out_d = nc.dram_tensor("out", [i, s], CHDT, kind="ExternalOutput").ap()

    with tile.TileContext(nc) as tc:
        from contextlib import ExitStack

        with ExitStack() as ctx:
            const_pool = ctx.enter_context(tc.tile_pool(name="const", bufs=1))
            xt_pool = ctx.enter_context(tc.tile_pool(name="xt", bufs=1))
            wst_pool = ctx.enter_context(tc.tile_pool(name="wstream", bufs=1))
            ps_pool = ctx.enter_context(
                tc.tile_pool(name="mmpsum", bufs=1, space="PSUM"))
            sb_pool = ctx.enter_context(tc.tile_pool(name="work", bufs=1))

            bi_t = const_pool.tile([P, ni], F32, name="bi_t")
            nc.sync.dma_start(bi_t[:], bi_d[:])
            bah_t = const_pool.tile([P, ni], F32, name="bah_t")
            nc.sync.dma_start(bah_t[:], bah_d[:])
            lnam_t = const_pool.tile([P, ni], F32, name="lnam_t")
            nc.sync.dma_start(lnam_t[:], lnam_d[:])

            act_chain = []

            def act(out_ap, in_ap, func, **kw):
                inst = nc.scalar.activation(out_ap, in_ap, func, **kw)
                if act_chain:
                    add_dep_helper(inst.ins, act_chain[-1].ins, False,
                                   "act table phase order")
                act_chain.append(inst)
                return inst

            # ---- weight stream: first cycles' tiles load BEFORE x ------
            w_sb_cache = {}

            def load_w(ic):
                if ic in w_sb_cache:
                    return w_sb_cache[ic]
                wi_sb = wst_pool.tile([P, d], MMDT, name=f"wi{ic}",
                                      tag="wi", bufs=3)
                nc.sync.dma_start(wi_sb[:], wiT_d[ic])
                wa_sb = wst_pool.tile([P, d], MMDT, name=f"wa{ic}",
                                      tag="wa", bufs=3)
                nc.sync.dma_start(wa_sb[:], waT_d[ic])
                w_sb_cache[ic] = (wi_sb, wa_sb)
                return w_sb_cache[ic]

            for ic in range(min(2, ni)):
                load_w(ic)

            # ---- resident x^T tiles, k-interleaved column loads --------
            xT_sb = []
            for k in range(nd):
                xT_sb.append(xt_pool.tile([P, s], MMDT, name=f"xT{k}"))
            # PE warm-up: ~10 wide matmuls on zeroed SBUF so the PE exits
            # its low-power pstate before the first real GEMM arrives.
            # Uses xT[0][:, :512] (overwritten by the x DMA afterwards)
            # and the pi-tag PSUM buffer (WAR-ordered before pi GEMMs).
            nc.vector.memset(xT_sb[0][:, 0:512], 0)
            warm_ps = ps_pool.tile([P, s], F32, name="warm", tag="pi",
                                   bufs=1)
            for _ in range(10):
                nc.tensor.matmul(warm_ps[:, 0:512], xT_sb[0][:, 0:128],
                                 xT_sb[0][:, 0:512], start=True, stop=True)
            xcw = min(512, s)
            for h in range(s // xcw):
                for k in range(nd):
                    nc.sync.dma_start(
                        xT_sb[k][:, h * xcw:(h + 1) * xcw],
                        xT_d[k * P:(k + 1) * P, h * xcw:(h + 1) * xcw])

            def gemm(ps, w_sb):
                for m in range(nmm):
                    for k in range(nd):
                        nc.tensor.matmul(
                            ps[:, m * 512:(m + 1) * 512],
                            w_sb[:, k * P:(k + 1) * P],
                            xT_sb[k][:, m * 512:(m + 1) * 512],
                            start=(k == 0), stop=(k == nd - 1))

            cycles = [list(range(c0, min(c0 + cyc, ni)))
                      for c0 in range(0, ni, cyc)]

            w_t, t_t, a_t, a2_t = {}, {}, {}, {}

            def phase1(ics, halves=False):
                # silu(z) = (z/2)*(1 + tanh(z/2)); the host pre-scales
                # Wi (and bi) by 0.5, so pi' = z/2 arrives from the GEMM
                # and w = (tanh(pi'+bi') + 1) * pi'' via one DVE STT.
                # tanh/exp share a table set -> the exp excursion costs
                # no table loads.
                for ic in ics:
                    wi_sb, wa_sb = load_w(ic)
                    pi_ps = ps_pool.tile([P, s], F32, name=f"pi{ic}",
                                         tag="pi", bufs=1)
                    gemm(pi_ps, wi_sb)
                    pa_ps = ps_pool.tile([P, s], F32, name=f"pa{ic}",
                                         tag="pa", bufs=1)
                    gemm(pa_ps, wa_sb)

                    if has_bi:
                        # generic-bias path: pib = pi' + bi' in SBUF
                        pib = sb_pool.tile([P, s], F32, name=f"pib{ic}",
                                           tag="pib", bufs=2)
                        nc.vector.tensor_scalar_add(pib[:], pi_ps[:],
                                                    bi_t[:, ic:ic + 1])
                        pi_src = pib
                    else:
                        pi_src = pi_ps

                    tp = sb_pool.tile([P, s], F32, name=f"tp{ic}", tag="tp",
                                      bufs=3)
                    act(tp[:], pi_src[:], AF.Tanh)
                    wt = sb_pool.tile([P, s], WQDT, name=f"w{ic}", tag="w",
                                      bufs=2 * cyc + 1)
                    nc.vector.scalar_tensor_tensor(
                        wt[:], tp[:], 1.0, pi_src[:],
                        op0=ALU.add, op1=ALU.mult)

                    tt = sb_pool.tile([P, s], F32, name=f"t{ic}", tag="t",
                                      bufs=2 * cyc)
                    act(tt[:], pa_ps[:], AF.Tanh, scale=0.5,
                        bias=bah_t[:, ic:ic + 1])
                    w_t[ic] = wt
                    t_t[ic] = tt

            def phase2(ics):
                for ic in ics:
                    at = sb_pool.tile([P, s], F32, name=f"a{ic}", tag="a",
                                      bufs=cyc + 2)
                    act(at[:], t_t[ic][:], AF.Exp, scale=-LN3 / 2.0,
                        bias=lnam_t[:, ic:ic + 1])
                    a_t[ic] = at
                    a2 = sb_pool.tile([P, s], F32, name=f"a2{ic}", tag="a2",
                                      bufs=cyc + 1)
                    eng = nc.gpsimd if a2_engine == "gp" else nc.vector
                    eng.tensor_mul(a2[:], at[:], at[:])
                    a2_t[ic] = a2

            def phase3(ics):
                for ic in ics:
                    qt = sb_pool.tile([P, s], WQDT, name=f"q{ic}", tag="q",
                                      bufs=3)
                    act(qt[:], a2_t[ic][:], AF.Sqrt, scale=-1.0, bias=1.0)
                    ct = sb_pool.tile([P, s], CHDT, name=f"c{ic}", tag="c",
                                      bufs=3)
                    eng = nc.gpsimd if wc_engine == "gp" else nc.vector
                    eng.tensor_mul(ct[:], qt[:], w_t[ic][:])
                    ht = sb_pool.tile([P, s], CHDT, name=f"h{ic}", tag="h",
                                      bufs=2)
                    nc.vector.tensor_tensor_scan(
                        ht[:], a_t[ic][:], ct[:], 0.0,
                        op0=ALU.mult, op1=ALU.add)
                    nc.sync.dma_start(out_d[ic * P:(ic + 1) * P, :], ht[:])

            # software pipeline with interleaved excursions: cycle k's
            # P1 chunks alternate with cycle k-1's exp / sqrt bursts, so
            # each ACT excursion (~7us) matches the one-chunk PSUM
            # runway of the PE and neither engine waits a full window.
            from collections import deque
            pending = deque()
            for k, ics in enumerate(cycles):
                if k >= 1:
                    pending.append(("exp", cycles[k - 1]))
                    pending.append(("sqrt", cycles[k - 1]))
                # with cyc=2: [c0] E [c1] S — every chunk's GEMM pair is
                # hidden behind an excursion; no unhidden serialization
                for ic in ics:
                    phase1([ic])
                    if pending:
                        kind, pics = pending.popleft()
                        (phase2 if kind == "exp" else phase3)(pics)
                while len(ics) < 2 and pending and k == len(cycles) - 1:
                    kind, pics = pending.popleft()
                    (phase2 if kind == "exp" else phase3)(pics)
            pending.append(("exp", cycles[-1]))
            pending.append(("sqrt", cycles[-1]))
            while pending:
                kind, pics = pending.popleft()
                (phase2 if kind == "exp" else phase3)(pics)

    nc.compile()
    return nc


@functools.lru_cache(maxsize=2)
def _get_nc(s=S, d=D, i=I, has_bi=False):
    return _build_nc(s, d, i, has_bi=has_bi)


LAST_RESULTS = None


def _to_mm_dtype(arr):
    if MM_DT == "bf16":
        import ml_dtypes
        return arr.astype(ml_dtypes.bfloat16)
    return np.ascontiguousarray(arr)  # f32r: raw f32 bits


def _prep_core_inputs(xb, WaT, WiT, biT, bahT, lnamT):
    return {"xT": _to_mm_dtype(np.ascontiguousarray(xb.T)), "WaT": WaT,
            "WiT": WiT, "biT": biT, "bahT": bahT, "lnamT": lnamT}


def _prep_shared(Wa, ba, Wi, bi, gate, d, i):
    ni = i // P
    nd = d // P
    WaT = _to_mm_dtype(
        Wa.reshape(ni, P, nd, P).transpose(0, 3, 2, 1).reshape(ni, P, d))
    WiT = _to_mm_dtype(
        (0.5 * Wi).reshape(ni, P, nd, P).transpose(0, 3, 2, 1).reshape(ni, P, d))
    biT = np.ascontiguousarray((0.5 * bi).reshape(ni, P).T)
    bahT = np.ascontiguousarray((0.5 * ba).reshape(ni, P).T)
    g64 = gate.astype(np.float64)
    lnam = (-np.log1p(np.exp(-g64)) - LN3 / 2.0).astype(np.float32)
    lnamT = np.ascontiguousarray(lnam.reshape(ni, P).T)
    return WaT, WiT, biT, bahT, lnamT


def kernel(x, Wa, ba, Wi, bi, gate):
    global LAST_RESULTS
    from concourse.bass_utils import run_bass_kernel_spmd

    x = np.asarray(x, dtype=np.float32)
    b, s, d = x.shape
    i = Wa.shape[0]
    nc = _get_nc(s, d, i, bool(np.any(np.asarray(bi) != 0)))

    WaT, WiT, biT, bahT, lnamT = _prep_shared(
        np.asarray(Wa, np.float32), np.asarray(ba, np.float32),
        np.asarray(Wi, np.float32), np.asarray(bi, np.float32),
        np.asarray(gate, np.float32), d, i)

    in_maps = [_prep_core_inputs(x[bb], WaT, WiT, biT, bahT, lnamT)
               for bb in range(b)]
    res = run_bass_kernel_spmd(nc, in_maps, list(range(b)))
    LAST_RESULTS = res
    out = np.stack(
        [np.asarray(res.results[bb]["out"], dtype=np.float32).T
         for bb in range(b)], axis=0)
    return np.ascontiguousarray(out, dtype=np.float32)
